# revision 1
# baseline (speedup 1.0000x reference)
"""MetaQuickSR Trainium2 kernel (8-core SPMD, row-sharded).

Sharding: H=256 output-feature rows split 32/core (+4-row conv halo).
Each core computes: 4-layer CNN -> implicit im2col -> Pos2Weight MLP ->
per-pixel locally-connected matmul -> its 64-row slab of the (4,3,512,512)
output.  No cross-core communication.
"""

import numpy as np
import ml_dtypes

import concourse.bass as bass
import concourse.mybir as mybir
from concourse.tile import TileContext
from concourse.bass_utils import run_bass_kernel_spmd
from concourse.dve_ops import TENSOR_TENSOR_REDUCE

BF16 = ml_dtypes.bfloat16

NCORES = 8
N, CI, Himg, Wimg, S = 4, 16, 256, 256, 2
ROWS = Himg // NCORES          # 32 output-feature rows per core
HALO = 4
NR = ROWS + 2 * HALO           # 40 buffered rows
WP = Wimg + 2                  # 258 zero-padded width
NPIX = ROWS * Wimg             # 8192 einsum pixels per core
NT = NPIX // 128               # 64 pixel tiles
PCH = 8                        # 1024-pixel chunks per q plane
RGB_MEAN = (0.4488, 0.4371, 0.404)
RGB_RANGE = 255.0

_NC = None


def _legalize_waits(nc, lim=1):
    """This walrus build accepts only one sync-wait per instruction; move
    surplus waits onto same-engine NoOps inserted just before."""
    cnt = 0
    for f in nc.m.functions:
        for bb in f.blocks:
            new = []
            for inst in bb.instructions:
                si = inst.sync_info
                if si is not None and si.on_wait is not None \
                        and len(si.on_wait) > lim:
                    waits = list(si.on_wait)
                    excess, keep = waits[:-lim], waits[-lim:]
                    for w in excess:
                        cnt += 1
                        nop = mybir.InstNoOp(
                            name=f"I-lw{cnt}", opcode="NoOp",
                            engine=inst.engine, debug=inst.debug,
                            ins=[], outs=[],
                            sync_info=mybir.SyncInfo(on_wait=[w],
                                                     on_update=[]))
                        new.append(nop)
                        nc.inst_map[nop.name] = nop
                    inst.sync_info = mybir.SyncInfo(
                        on_wait=keep, on_update=list(si.on_update or []))
                new.append(inst)
            bb.instructions = new
    return cnt


def _build_program():
    nc = bass.Bass(trn_type="TRN2")
    f32 = mybir.dt.float32
    bf = mybir.dt.bfloat16

    # packed constant inputs: [x | cw | w2p] bf16,
    # [w1 | cb | b1c | b2p | ones | mean-shift] f32
    BFW = NR * WP + 4 * 9 * 16 + 2 * 432          # 11760
    FW = 256 + 4 + 2 + 432 + 128 + NT * 12        # 1590
    bfin = nc.dram_tensor("bfin", [128, BFW], bf, kind="ExternalInput")
    f32in = nc.dram_tensor("f32in", [128, FW], f32, kind="ExternalInput")
    post = nc.dram_tensor("post", [4, 3, NPIX], f32, kind="ExternalInput")
    outd = nc.dram_tensor("out", [4, 3, 2 * ROWS, 2 * Wimg], f32,
                          kind="ExternalOutput")

    with TileContext(nc) as tc:
        with (
            tc.tile_pool(name="singles", bufs=1) as singles,
            tc.tile_pool(name="pos_p", bufs=2) as pos_p,
            tc.tile_pool(name="ht_p", bufs=2) as ht_p,
            tc.tile_pool(name="lws_p", bufs=3) as lws_p,
            tc.tile_pool(name="scr_p", bufs=2) as scr_p,
            tc.tile_pool(name="cps", bufs=2, space="PSUM") as cps,
            tc.tile_pool(name="hps", bufs=2, space="PSUM") as hps,
            tc.tile_pool(name="lps", bufs=2, space="PSUM") as lps,
        ):
            # ---- resident inputs -------------------------------------
            bf_sb = singles.tile([128, BFW], bf)
            f32_sb = singles.tile([128, FW], f32)
            fA = singles.tile([128, NR, WP], bf)
            fB = singles.tile([128, NR, WP], bf)
            f4c = singles.tile([64, NR, WP], bf)
            # fT2h[half][p, (row,kw), (n,ci)]: transposed f4 rows 3..36 with
            # 3 horizontal shifts; a tile's 9 tap blocks are equally spaced
            # (tap stride 64) so one image's patch is a 2-free-dim AP.
            fT2h = [singles.tile([128, 34 * 3 * 64], bf, name=f"fT2h{h}")
                    for h in range(2)]
            outq = [singles.tile([128, NT * 12], f32, name=f"outq{q}")
                    for q in range(4)]
            dummy = singles.tile([1, 16], bf)

            nc.scalar.dma_start(bf_sb[:, :], bfin[:, :])
            nc.scalar.dma_start(f32_sb[:, :], f32in[:, :])
            nc.gpsimd.memset(fA[:, :, :], 0.0)
            nc.gpsimd.memset(fB[:, :, :], 0.0)

            # warm ACT's vector clock (1 wait per op) so conv relu-copies
            # only ever wait on PE.
            nc.scalar.copy(dummy[0:1, 0:1], bf_sb[0:1, 0:1])
            nc.scalar.copy(dummy[0:1, 1:2], f32_sb[0:1, 0:1])
            nc.scalar.copy(dummy[0:1, 2:3], fA[0:1, 0:1, 0:1])
            nc.scalar.copy(dummy[0:1, 3:4], fB[0:1, 0:1, 0:1])

            x_sb = bf_sb[:, 0:NR * WP].rearrange("p (r w) -> p r w", w=WP)
            cw_sb = bf_sb[:, NR * WP:NR * WP + 576].rearrange(
                "p (l t o) -> p l t o", t=9, o=16)
            w2p_sb = bf_sb[:, NR * WP + 576:].rearrange(
                "p (j c) -> p j c", c=432)
            w1_sb = f32_sb[0:3, 0:256]
            cb_sb = f32_sb[:, 256:260]
            b1_sb = f32_sb[:, 260:262]
            b2p_sb = f32_sb[0:1, 262:694]
            ones_sb = f32_sb[0:1, 694:822]
            shift_sb = f32_sb[:, 822:822 + NT * 12]

            # ---- conv chain ------------------------------------------
            # l: 0:x->fA  1:fA->fB  2:fB->fA  3:fA->fB, then fB->f4c
            fins = [x_sb, fA, fB, fA]
            fouts = [fA, fB, fA, fB]
            for l in range(4):
                K = 3 if l == 0 else 16
                fin, fout = fins[l], fouts[l]
                for ch in range(19):
                    r0 = 1 + 2 * ch
                    ps = cps.tile([128, 2, 256], f32, tag="convps")
                    for tap in range(9):
                        kh, kw = tap // 3, tap % 3
                        for n in range(4):
                            nc.tensor.matmul(
                                ps[32 * n:32 * n + 16, :, :],
                                cw_sb[32 * n:32 * n + K, l, tap, :],
                                fin[32 * n:32 * n + K,
                                    r0 + kh - 1:r0 + kh + 1,
                                    kw:kw + 256],
                                start=(tap == 0), stop=(tap == 8),
                                tile_position=(32 * n, 32 * n),
                            )
                    nc.scalar.activation(
                        fout[:, r0:r0 + 2, 1:257], ps[:, :, :],
                        mybir.ActivationFunctionType.Relu,
                        bias=cb_sb[:, l:l + 1], scale=1.0)

            # compact (32n+ci) -> contiguous 64 partitions for the xbar
            for n in range(4):
                nc.scalar.dma_start(
                    out=f4c[16 * n:16 * n + 16, :, :],
                    in_=fB[32 * n:32 * n + 16, :, :])

            # warm SP's clock on the 4 compaction DMAs (1 wait each)
            for n in range(4):
                nc.sync.dma_start(out=dummy[0:1, 4 + n:5 + n],
                                  in_=f4c[16 * n:16 * n + 1, 0:1, 0:1])

            # ---- im2col: shared row-transpose cache ------------------
            for r in range(34):
                for hf in range(2):
                    for kw in range(3):
                        nc.sync.dma_start_transpose(
                            out=fT2h[hf][:, (3 * r + kw) * 64:
                                         (3 * r + kw + 1) * 64],
                            in_=f4c[:, r + 3, 128 * hf + kw:
                                    128 * hf + kw + 128])
            fT2v = [t.rearrange("p (t x) -> p t x", x=64) for t in fT2h]

            # warm DVE's clock across the HWDGE sem lanes (recent
            # transposes cover the lanes; 1 wait per touch op).
            for k in range(9):
                r, hf, kw = 33 - (k // 6), (k // 3) % 2, k % 3
                nc.vector.tensor_copy(dummy[0:1, 13 + k % 3:14 + k % 3],
                                      fT2v[hf][0:1, 3 * r + kw, 0:1])

            # ---- per-q: h MLP, local weights, einsum -----------------
            mul, add = mybir.AluOpType.mult, mybir.AluOpType.add
            for q in range(4):
                for pc in range(PCH):
                    pos_t = pos_p.tile([3, 1024], f32, tag="pos")
                    nc.scalar.dma_start(
                        pos_t[:, :], post[q, :, pc * 1024:(pc + 1) * 1024])
                    hT = ht_p.tile([128, 2, 1024], bf, tag="ht")
                    for jh in range(2):
                        for hf in range(2):
                            hp = hps.tile([128, 512], f32, tag="hps")
                            nc.tensor.matmul(
                                hp[:, :],
                                w1_sb[:, jh * 128:(jh + 1) * 128],
                                pos_t[:, hf * 512:(hf + 1) * 512],
                                start=True, stop=True)
                            nc.scalar.activation(
                                hT[:, jh, hf * 512:(hf + 1) * 512], hp[:, :],
                                mybir.ActivationFunctionType.Relu,
                                bias=b1_sb[:, jh:jh + 1], scale=1.0)
                    for tl in range(8):
                        t = pc * 8 + tl
                        r0, hf = t // 2, t % 2
                        lwp = lps.tile([128, 3, 9, 16], f32, tag="lwp")
                        for jh in range(2):
                            nc.tensor.matmul(
                                lwp[:, :, :, :],
                                hT[:, jh, tl * 128:(tl + 1) * 128],
                                w2p_sb[:, jh, :],
                                start=(jh == 0), stop=False)
                        nc.tensor.matmul(
                            lwp[:, :, :, :], ones_sb[:, :], b2p_sb[:, :],
                            start=False, stop=True)
                        lws = lws_p.tile([128, 3, 9, 16], bf, tag="lws")
                        nc.scalar.activation(
                            lws[:, :, :, :], lwp[:, :, :, :],
                            mybir.ActivationFunctionType.Copy)
                        for n in range(4):
                            for c in range(3):
                                scr = scr_p.tile([128, 9, 16], bf,
                                                 tag="scr")
                                nc.vector.scalar_tensor_tensor(
                                    out=scr[:, :, :],
                                    in0=fT2v[hf][:, 3 * r0:3 * r0 + 9,
                                                 n * 16:(n + 1) * 16],
                                    scalar=1.0,
                                    in1=lws[:, c, :, :],
                                    op0=mul, op1=mul,
                                    accum_out=outq[q][:, (n * 3 + c) * NT + t:
                                                      (n * 3 + c) * NT + t + 1])
                # add_mean: +255*RGB_MEAN[c] to every output element
                nc.vector.tensor_add(outq[q][:, :], outq[q][:, :],
                                     shift_sb)

            # ---- writeback -------------------------------------------
            for q in range(4):
                si, sj = q // 2, q % 2
                dstv = outd.rearrange(
                    "n c (h2 s1) (w1 w2 s2) -> n c s1 s2 w2 h2 w1",
                    s1=2, w1=2, s2=2)
                for n in range(4):
                    for c in range(3):
                        nci = n * 3 + c
                        src = outq[q][:, nci * NT:(nci + 1) * NT].rearrange(
                            "p (t2 t1) -> p t2 t1", t1=2)
                        for w1 in range(2):
                            nc.gpsimd.dma_start(
                                out=dstv[n, c, si, sj, :, :, w1],
                                in_=src[:, :, w1])
    _legalize_waits(nc)
    return nc


def _get_nc():
    global _NC
    if _NC is None:
        _NC = _build_program()
    return _NC


def _prep_inputs(x, pos_mat, c0w, c0b, c1w, c1b, c2w, c2b, c3w, c3b,
                 w1, b1, w2, b2):
    """Host-side packing of per-core input dicts."""
    x = np.asarray(x, np.float32)
    pos = np.asarray(pos_mat, np.float32).reshape(-1, 3)

    # conv weights: cw[32n+ci, l, kh*3+kw, co]
    cwp = np.zeros((128, 4, 9, 16), np.float32)
    cbp = np.zeros((128, 4), np.float32)
    for l, (wl, bl) in enumerate(((c0w, c0b), (c1w, c1b),
                                  (c2w, c2b), (c3w, c3b))):
        wl = np.asarray(wl, np.float32)          # (co, ci, 3, 3)
        K = wl.shape[1]
        t = wl.transpose(1, 2, 3, 0).reshape(K, 9, 16)   # (ci, tap, co)
        for n in range(4):
            cwp[32 * n:32 * n + K, l] = t
            cbp[32 * n:32 * n + 16, l] = np.asarray(bl, np.float32)

    w1 = np.asarray(w1, np.float32)              # (3, 256)
    b1p = np.asarray(b1, np.float32).reshape(2, 128).T.copy()  # [j, jh]

    # w2 columns: orig (s=ci*9+tap, c) -> permuted (c, tap, ci)
    w2 = np.asarray(w2, np.float32).reshape(256, 16, 9, 3)     # j, ci, tap, c
    w2pm = w2.transpose(0, 3, 2, 1).reshape(256, 432)          # j,(c,tap,ci)
    w2pk = w2pm.reshape(2, 128, 432).astype(BF16)              # [jh, j, 432]
    w2pk = np.ascontiguousarray(w2pk.transpose(1, 0, 2))       # [j, jh, 432]
    b2 = np.asarray(b2, np.float32).reshape(16, 9, 3)
    b2pk = b2.transpose(2, 1, 0).reshape(1, 432)

    # pos rows ordered (h, si, w, sj); per-core chunk -> (q, 3, NPIX)
    posr = pos.reshape(Himg, 2, Wimg, 2, 3)

    # f32 pack: [w1 | cb | b1c | b2p | ones | mean-shift]
    f32pk = np.zeros((128, 822 + NT * 12), np.float32)
    f32pk[0:3, 0:256] = w1
    f32pk[:, 256:260] = cbp
    f32pk[:, 260:262] = b1p
    f32pk[0, 262:694] = b2pk[0]
    f32pk[0, 694:822] = 1.0
    shift = np.zeros(NT * 12, np.float32)
    for n in range(4):
        for c in range(3):
            shift[(n * 3 + c) * NT:(n * 3 + c + 1) * NT] = \
                RGB_RANGE * RGB_MEAN[c]
    f32pk[:, 822:] = shift

    in_maps = []
    for core in range(NCORES):
        h0 = core * ROWS
        xh = np.zeros((128, NR, WP), np.float32)
        lo, hi = h0 - HALO, h0 + ROWS + HALO
        slo, shi = max(lo, 0), min(hi, Himg)
        for n in range(4):
            xh[32 * n:32 * n + 3, slo - lo:shi - lo, 1:257] = \
                x[n, :, slo:shi, :]
        bfpk = np.concatenate(
            [xh.reshape(128, -1), cwp.reshape(128, -1),
             w2pk.reshape(128, -1).astype(np.float32)], axis=1)
        pc = posr[h0:h0 + ROWS].transpose(1, 3, 4, 0, 2)  # si,sj,3,h,w
        pc = pc.reshape(2, 2, 3, NPIX).reshape(4, 3, NPIX)
        in_maps.append({
            "bfin": bfpk.astype(BF16),
            "f32in": f32pk,
            "post": np.ascontiguousarray(pc),
        })
    return in_maps


LAST_RESULTS = None
TRACE = False


def kernel(**inputs):
    global LAST_RESULTS
    nc = _get_nc()
    in_maps = _prep_inputs(**inputs)
    res = run_bass_kernel_spmd(nc, in_maps, core_ids=list(range(NCORES)),
                               trace=TRACE)
    LAST_RESULTS = res
    out = np.concatenate([res.results[i]["out"] for i in range(NCORES)],
                         axis=2)
    return out.astype(np.float32)



# revision 3
# speedup vs baseline: 2.1223x; 2.1223x over previous
"""MetaQuickSR Trainium2 kernel (8-core SPMD, row-sharded).

Sharding: H=256 output-feature rows split 32/core (+4-row conv halo).
Each core computes: 4-layer CNN -> implicit im2col -> Pos2Weight MLP ->
per-pixel locally-connected matmul -> its 64-row slab of the (4,3,512,512)
output.  No cross-core communication.
"""

import numpy as np
import ml_dtypes

import concourse.bass as bass
import concourse.mybir as mybir
from concourse.tile import TileContext
from concourse.bass_utils import run_bass_kernel_spmd

BF16 = ml_dtypes.bfloat16

NCORES = 8
N, CI, Himg, Wimg, S = 4, 16, 256, 256, 2
ROWS = Himg // NCORES          # 32 output-feature rows per core
HALO = 4
NR = ROWS + 2 * HALO           # 40 buffered rows
WP = Wimg + 2                  # 258 zero-padded width
NPIX = ROWS * Wimg             # 8192 einsum pixels per core
NT = NPIX // 128               # 64 pixel tiles
PCH = 8                        # 1024-pixel chunks per q plane
RGB_MEAN = (0.4488, 0.4371, 0.404)
RGB_RANGE = 255.0

# bf16-pack column offsets
_XW = NR * WP                  # 10320
_CWO = _XW                     # conv weights (4*9*16 = 576)
_W2O = _CWO + 576              # w2 permuted (2*432 = 864)
_W1O = _W2O + 864              # w1 bf16 (256, rows 0-2)
_B2O = _W1O + 256              # b2 permuted bf16 (432, row 0)
_ONO = _B2O + 432              # ones bf16 (128, row 0)
BFW = _ONO + 128               # 12576
FW = 4 + 2 + NT * 12           # f32 pack: cb | b1c | shift

_NC = None


def _legalize_waits(nc, lim=1):
    """This walrus build accepts only one sync-wait per instruction; move
    surplus waits onto same-engine NoOps inserted just before."""
    cnt = 0
    for f in nc.m.functions:
        for bb in f.blocks:
            new = []
            for inst in bb.instructions:
                si = inst.sync_info
                if si is not None and si.on_wait is not None \
                        and len(si.on_wait) > lim:
                    waits = list(si.on_wait)
                    excess, keep = waits[:-lim], waits[-lim:]
                    for w in excess:
                        cnt += 1
                        nop = mybir.InstNoOp(
                            name=f"I-lw{cnt}", opcode="NoOp",
                            engine=inst.engine, debug=inst.debug,
                            ins=[], outs=[],
                            sync_info=mybir.SyncInfo(on_wait=[w],
                                                     on_update=[]))
                        new.append(nop)
                        nc.inst_map[nop.name] = nop
                    inst.sync_info = mybir.SyncInfo(
                        on_wait=keep, on_update=list(si.on_update or []))
                new.append(inst)
            bb.instructions = new
    return cnt


def _build_program():
    nc = bass.Bass(trn_type="TRN2")
    f32 = mybir.dt.float32
    bf = mybir.dt.bfloat16

    bfin = nc.dram_tensor("bfin", [128, BFW], bf, kind="ExternalInput")
    f32in = nc.dram_tensor("f32in", [128, FW], f32, kind="ExternalInput")
    post = nc.dram_tensor("post", [4, 3, NPIX], bf, kind="ExternalInput")
    outd = nc.dram_tensor("out", [4, 128, NT * 12], f32,
                          kind="ExternalOutput")

    with TileContext(nc) as tc:
        with (
            tc.tile_pool(name="singles", bufs=1) as singles,
            tc.tile_pool(name="pos_p", bufs=2) as pos_p,
            tc.tile_pool(name="ht_p", bufs=2) as ht_p,
            tc.tile_pool(name="lws_p", bufs=3) as lws_p,
            tc.tile_pool(name="prod_p", bufs=2) as prod_p,
            tc.tile_pool(name="cps", bufs=2, space="PSUM") as cps,
            tc.tile_pool(name="hps", bufs=2, space="PSUM") as hps,
            tc.tile_pool(name="lps", bufs=2, space="PSUM") as lps,
            tc.tile_pool(name="wps_p", bufs=1, space="PSUM") as wps_p,
        ):
            # ---- resident inputs -------------------------------------
            bf_sb = singles.tile([128, BFW], bf)
            f32_sb = singles.tile([128, FW], f32)
            fA = singles.tile([128, NR, WP], bf)
            fB = singles.tile([128, NR, WP], bf)
            f4c = singles.tile([64, NR, WP], bf)
            wsrc = singles.tile([128, 512], bf)
            # fT2h[half][p, (row,kw), (n,ci)]: transposed f4 rows 3..36 with
            # 3 horizontal shifts; a tile's 9 tap blocks are equally spaced
            # (tap stride 64) so one image's patch is a 2-free-dim AP.
            fT2h = [singles.tile([128, 34 * 3 * 64], bf, name=f"fT2h{h}")
                    for h in range(2)]
            outq = [singles.tile([128, NT * 12], f32, name=f"outq{q}")
                    for q in range(4)]
            dummy = singles.tile([1, 16], bf)

            nc.scalar.dma_start(bf_sb[:, :], bfin[:, :])
            nc.scalar.dma_start(f32_sb[:, :], f32in[:, :])
            nc.gpsimd.memset(wsrc[:, :], 1.0)
            nc.gpsimd.memset(fA[:, :, :], 0.0)
            nc.gpsimd.memset(fB[:, :, :], 0.0)

            # HAM warm-up: dense full-array matmuls on dummy data so the PE
            # clock gate opens (cold 1.2 GHz -> warm 2.4 GHz) before and
            # during the quadrant-packed conv (masked MMs may not register
            # as PE activity).
            wps = wps_p.tile([128, 512], f32)
            for i in range(24):
                nc.tensor.matmul(wps[:, :], wsrc[:, 0:128], wsrc[:, :],
                                 start=True, stop=True)

            # warm ACT's vector clock (1 wait per op) so conv relu-copies
            # only ever wait on PE.
            nc.scalar.copy(dummy[0:1, 0:1], bf_sb[0:1, 0:1])
            nc.scalar.copy(dummy[0:1, 1:2], f32_sb[0:1, 0:1])
            nc.scalar.copy(dummy[0:1, 2:3], fA[0:1, 0:1, 0:1])
            nc.scalar.copy(dummy[0:1, 3:4], fB[0:1, 0:1, 0:1])

            x_sb = bf_sb[:, 0:_XW].rearrange("p (r w) -> p r w", w=WP)
            cw_sb = bf_sb[:, _CWO:_CWO + 576].rearrange(
                "p (l t o) -> p l t o", t=9, o=16)
            w2p_sb = bf_sb[:, _W2O:_W2O + 864].rearrange(
                "p (j c) -> p j c", c=432)
            w1_sb = bf_sb[0:3, _W1O:_W1O + 256]
            b2p_sb = bf_sb[0:1, _B2O:_B2O + 432]
            ones_sb = bf_sb[0:1, _ONO:_ONO + 128]
            cb_sb = f32_sb[:, 0:4]
            b1_sb = f32_sb[:, 4:6]
            shift_sb = f32_sb[:, 6:6 + NT * 12]

            # ---- conv chain ------------------------------------------
            # l: 0:x->fA  1:fA->fB  2:fB->fA  3:fA->fB, then fB->f4c
            fins = [x_sb, fA, fB, fA]
            fouts = [fA, fB, fA, fB]
            for l in range(4):
                K = 3 if l == 0 else 16
                fin, fout = fins[l], fouts[l]
                for ch in range(19):
                    r0 = 1 + 2 * ch
                    ps = cps.tile([128, 2, 256], f32, tag="convps")
                    for tap in range(9):
                        kh, kw = tap // 3, tap % 3
                        for n in range(4):
                            nc.tensor.matmul(
                                ps[32 * n:32 * n + 16, :, :],
                                cw_sb[32 * n:32 * n + K, l, tap, :],
                                fin[32 * n:32 * n + K,
                                    r0 + kh - 1:r0 + kh + 1,
                                    kw:kw + 256],
                                start=(tap == 0), stop=(tap == 8),
                                tile_position=(32 * n, 32 * n),
                                # CoreSim's zero-region model rejects the 4
                                # partition-disjoint concurrent groups; HW
                                # has_written is per-element (baseline-
                                # validated on HW).
                                skip_group_check=True,
                            )
                    # keep-warm pulse: one unmasked full-array MM per chunk
                    nc.tensor.matmul(wps[:, 0:64], wsrc[:, 0:128],
                                     wsrc[:, 0:64], start=True, stop=True)
                    nc.scalar.activation(
                        fout[:, r0:r0 + 2, 1:257], ps[:, :, :],
                        mybir.ActivationFunctionType.Relu,
                        bias=cb_sb[:, l:l + 1], scale=1.0)

            # compact (32n+ci) -> contiguous 64 partitions for the xbar
            for n in range(4):
                nc.scalar.dma_start(
                    out=f4c[16 * n:16 * n + 16, :, :],
                    in_=fB[32 * n:32 * n + 16, :, :])

            # warm SP's clock on the 4 compaction DMAs (1 wait each)
            for n in range(4):
                nc.sync.dma_start(out=dummy[0:1, 4 + n:5 + n],
                                  in_=f4c[16 * n:16 * n + 1, 0:1, 0:1])

            # ---- im2col: shared row-transpose cache ------------------
            for r in range(34):
                for hf in range(2):
                    for kw in range(3):
                        nc.sync.dma_start_transpose(
                            out=fT2h[hf][:, (3 * r + kw) * 64:
                                         (3 * r + kw + 1) * 64],
                            in_=f4c[:, r + 3, 128 * hf + kw:
                                    128 * hf + kw + 128])
            fT2v = [t.rearrange("p (t x) -> p t x", x=64) for t in fT2h]

            # warm DVE's clock across the HWDGE sem lanes (recent
            # transposes cover the lanes; 1 wait per touch op).
            for k in range(9):
                r, hf, kw = 33 - (k // 6), (k // 3) % 2, k % 3
                nc.vector.tensor_copy(dummy[0:1, 13 + k % 3:14 + k % 3],
                                      fT2v[hf][0:1, 3 * r + kw, 0:1])

            # ---- per-q: h MLP, local weights, einsum -----------------
            mul, add = mybir.AluOpType.mult, mybir.AluOpType.add
            for q in range(4):
                outq_v = outq[q].rearrange("p (n c t) -> p c n t", n=4, c=3)
                for pc in range(PCH):
                    pos_t = pos_p.tile([3, 1024], bf, tag="pos")
                    nc.scalar.dma_start(
                        pos_t[:, :], post[q, :, pc * 1024:(pc + 1) * 1024])
                    hT = ht_p.tile([128, 2, 1024], bf, tag="ht")
                    for jh in range(2):
                        for hf in range(2):
                            hp = hps.tile([128, 512], f32, tag="hps")
                            nc.tensor.matmul(
                                hp[:, :],
                                w1_sb[:, jh * 128:(jh + 1) * 128],
                                pos_t[:, hf * 512:(hf + 1) * 512],
                                start=True, stop=True)
                            nc.scalar.activation(
                                hT[:, jh, hf * 512:(hf + 1) * 512], hp[:, :],
                                mybir.ActivationFunctionType.Relu,
                                bias=b1_sb[:, jh:jh + 1], scale=1.0)
                    for tl in range(8):
                        t = pc * 8 + tl
                        r0, hf = t // 2, t % 2
                        lwp = lps.tile([128, 3, 9, 16], f32, tag="lwp")
                        for jh in range(2):
                            nc.tensor.matmul(
                                lwp[:, :, :, :],
                                hT[:, jh, tl * 128:(tl + 1) * 128],
                                w2p_sb[:, jh, :],
                                start=(jh == 0), stop=False)
                        nc.tensor.matmul(
                            lwp[:, :, :, :], ones_sb[:, :], b2p_sb[:, :],
                            start=False, stop=True)
                        lws = lws_p.tile([128, 3, 9, 16], bf, tag="lws")
                        nc.scalar.activation(
                            lws[:, :, :, :], lwp[:, :, :, :],
                            mybir.ActivationFunctionType.Copy)
                        # products then per-(c,n) segment reduce
                        in0v = fT2v[hf][:, 3 * r0:3 * r0 + 9, :].rearrange(
                            "p t (n x) -> p n t x", n=4)
                        prod = prod_p.tile([128, 3, 4, 9, 16], bf,
                                           tag="prod")
                        for c in range(3):
                            nc.vector.tensor_tensor(
                                prod[:, c, :, :, :], in0v,
                                lws[:, c, :, :].unsqueeze(1).broadcast_to(
                                    (128, 4, 9, 16)),
                                mul)
                        nc.vector.tensor_reduce(
                            out=outq_v[:, :, :, t],
                            in_=prod.rearrange("p c n t x -> p c n (t x)"),
                            axis=mybir.AxisListType.X, op=add)
                # add_mean: +255*RGB_MEAN[c] to every output element
                nc.vector.tensor_add(outq[q][:, :], outq[q][:, :],
                                     shift_sb)

            # ---- writeback -------------------------------------------
            for q in range(4):
                nc.gpsimd.dma_start(out=outd[q], in_=outq[q][:, :])
    _legalize_waits(nc)
    return nc


def _get_nc():
    global _NC
    if _NC is None:
        _NC = _build_program()
    return _NC


def _prep_inputs(x, pos_mat, c0w, c0b, c1w, c1b, c2w, c2b, c3w, c3b,
                 w1, b1, w2, b2):
    """Host-side packing of per-core input dicts."""
    x = np.asarray(x, np.float32)
    pos = np.asarray(pos_mat, np.float32).reshape(-1, 3)

    # conv weights: cw[32n+ci, l, kh*3+kw, co]
    cwp = np.zeros((128, 4, 9, 16), np.float32)
    cbp = np.zeros((128, 4), np.float32)
    for l, (wl, bl) in enumerate(((c0w, c0b), (c1w, c1b),
                                  (c2w, c2b), (c3w, c3b))):
        wl = np.asarray(wl, np.float32)          # (co, ci, 3, 3)
        K = wl.shape[1]
        t = wl.transpose(1, 2, 3, 0).reshape(K, 9, 16)   # (ci, tap, co)
        for n in range(4):
            cwp[32 * n:32 * n + K, l] = t
            cbp[32 * n:32 * n + 16, l] = np.asarray(bl, np.float32)

    w1 = np.asarray(w1, np.float32)              # (3, 256)
    b1p = np.asarray(b1, np.float32).reshape(2, 128).T.copy()  # [j, jh]

    # w2 columns: orig (s=ci*9+tap, c) -> permuted (c, tap, ci)
    w2 = np.asarray(w2, np.float32).reshape(256, 16, 9, 3)     # j, ci, tap, c
    w2pm = w2.transpose(0, 3, 2, 1).reshape(256, 432)          # j,(c,tap,ci)
    w2pk = w2pm.reshape(2, 128, 432).astype(BF16)              # [jh, j, 432]
    w2pk = np.ascontiguousarray(w2pk.transpose(1, 0, 2))       # [j, jh, 432]
    b2 = np.asarray(b2, np.float32).reshape(16, 9, 3)
    b2pk = b2.transpose(2, 1, 0).reshape(432)                  # (c, tap, ci)

    # pos rows ordered (h, si, w, sj); per-core chunk -> (q, 3, NPIX)
    posr = pos.reshape(Himg, 2, Wimg, 2, 3)

    # bf16 tail shared by all cores: w1 | b2p | ones
    w1pad = np.zeros((128, 256), np.float32)
    w1pad[0:3] = w1
    b2pad = np.zeros((128, 432), np.float32)
    b2pad[0] = b2pk
    onespad = np.zeros((128, 128), np.float32)
    onespad[0] = 1.0

    # f32 pack: [cb | b1c | mean-shift]
    f32pk = np.zeros((128, FW), np.float32)
    f32pk[:, 0:4] = cbp
    f32pk[:, 4:6] = b1p
    shift = np.zeros(NT * 12, np.float32)
    for n in range(4):
        for c in range(3):
            shift[(n * 3 + c) * NT:(n * 3 + c + 1) * NT] = \
                RGB_RANGE * RGB_MEAN[c]
    f32pk[:, 6:] = shift

    in_maps = []
    for core in range(NCORES):
        h0 = core * ROWS
        xh = np.zeros((128, NR, WP), np.float32)
        lo, hi = h0 - HALO, h0 + ROWS + HALO
        slo, shi = max(lo, 0), min(hi, Himg)
        for n in range(4):
            xh[32 * n:32 * n + 3, slo - lo:shi - lo, 1:257] = \
                x[n, :, slo:shi, :]
        bfpk = np.concatenate(
            [xh.reshape(128, -1), cwp.reshape(128, -1),
             w2pk.reshape(128, -1).astype(np.float32),
             w1pad, b2pad, onespad], axis=1)
        pc = posr[h0:h0 + ROWS].transpose(1, 3, 4, 0, 2)  # si,sj,3,h,w
        pc = pc.reshape(2, 2, 3, NPIX).reshape(4, 3, NPIX)
        in_maps.append({
            "bfin": bfpk.astype(BF16),
            "f32in": f32pk,
            "post": np.ascontiguousarray(pc).astype(BF16),
        })
    return in_maps


def _unpack_core_output(raw):
    """[4(q), 128(p), 12*NT] f32 -> (4, 3, 2*ROWS, 2*Wimg)."""
    a = np.asarray(raw, np.float32).reshape(2, 2, 128, 4, 3, NT // 2, 2)
    # (si, sj, p, n, c, r0, hf) -> (n, c, r0, si, hf, p, sj)
    return a.transpose(3, 4, 5, 0, 6, 2, 1).reshape(4, 3, 2 * ROWS,
                                                    2 * Wimg)


LAST_RESULTS = None
TRACE = False


def kernel(**inputs):
    global LAST_RESULTS
    nc = _get_nc()
    in_maps = _prep_inputs(**inputs)
    res = run_bass_kernel_spmd(nc, in_maps, core_ids=list(range(NCORES)),
                               trace=TRACE)
    LAST_RESULTS = res
    out = np.concatenate(
        [_unpack_core_output(res.results[i]["out"]) for i in range(NCORES)],
        axis=2)
    return out.astype(np.float32)


# revision 8
# speedup vs baseline: 2.5589x; 1.2057x over previous
"""MetaQuickSR Trainium2 kernel (8-core SPMD, row-sharded).

Sharding: H=256 output-feature rows split 32/core (+4-row conv halo).
Each core computes: 4-layer CNN -> implicit im2col -> Pos2Weight MLP ->
per-pixel locally-connected matmul -> its 64-row slab of the (4,3,512,512)
output.  No cross-core communication.
"""

import numpy as np
import ml_dtypes

import concourse.bass as bass
import concourse.mybir as mybir
from concourse.tile import TileContext
from concourse.bass_utils import run_bass_kernel_spmd

BF16 = ml_dtypes.bfloat16

NCORES = 8
N, CI, Himg, Wimg, S = 4, 16, 256, 256, 2
ROWS = Himg // NCORES          # 32 output-feature rows per core
HALO = 4
NR = ROWS + 2 * HALO           # 40 buffered rows
WP = Wimg + 2                  # 258 zero-padded width
NPIX = ROWS * Wimg             # 8192 einsum pixels per core
NT = NPIX // 128               # 64 pixel tiles
PCH = 8                        # 1024-pixel chunks per q plane
RGB_MEAN = (0.4488, 0.4371, 0.404)
RGB_RANGE = 255.0

# bf16-pack column offsets
_XW = NR * WP                  # 10320
_CWO = _XW                     # conv weights (4*9*16 = 576)
_W2O = _CWO + 576              # w2 permuted (2*432 = 864)
_W1O = _W2O + 864              # w1 bf16 (256, rows 0-2)
_B2O = _W1O + 256              # b2 permuted bf16 (432, row 0)
_ONO = _B2O + 432              # ones bf16 (128, row 0)
BFW = _ONO + 128               # 12576
FW = 4 + 2 + NT * 12           # f32 pack: cb | b1c | shift

_NC = None


def _legalize_waits(nc, lim=1):
    """This walrus build accepts only one sync-wait per instruction; move
    surplus waits onto same-engine NoOps inserted just before."""
    cnt = 0
    for f in nc.m.functions:
        for bb in f.blocks:
            new = []
            for inst in bb.instructions:
                si = inst.sync_info
                if si is not None and si.on_wait is not None \
                        and len(si.on_wait) > lim:
                    waits = list(si.on_wait)
                    excess, keep = waits[:-lim], waits[-lim:]
                    for w in excess:
                        cnt += 1
                        nop = mybir.InstNoOp(
                            name=f"I-lw{cnt}", opcode="NoOp",
                            engine=inst.engine, debug=inst.debug,
                            ins=[], outs=[],
                            sync_info=mybir.SyncInfo(on_wait=[w],
                                                     on_update=[]))
                        new.append(nop)
                        nc.inst_map[nop.name] = nop
                    inst.sync_info = mybir.SyncInfo(
                        on_wait=keep, on_update=list(si.on_update or []))
                new.append(inst)
            bb.instructions = new
    return cnt


def _build_program():
    nc = bass.Bass(trn_type="TRN2")
    f32 = mybir.dt.float32
    bf = mybir.dt.bfloat16

    bfin = nc.dram_tensor("bfin", [128, BFW], bf, kind="ExternalInput")
    f32in = nc.dram_tensor("f32in", [128, FW], f32, kind="ExternalInput")
    post = nc.dram_tensor("post", [4, 3, NPIX], bf, kind="ExternalInput")
    outd = nc.dram_tensor("out", [4, 128, NT * 12], f32,
                          kind="ExternalOutput")

    with TileContext(nc) as tc:
        with (
            tc.tile_pool(name="singles", bufs=1) as singles,
            tc.tile_pool(name="pos_p", bufs=2) as pos_p,
            tc.tile_pool(name="ht_p", bufs=2) as ht_p,
            tc.tile_pool(name="lws_p", bufs=3) as lws_p,
            tc.tile_pool(name="prod_p", bufs=2) as prod_p,
            tc.tile_pool(name="cps", bufs=2, space="PSUM") as cps,
            tc.tile_pool(name="hps", bufs=2, space="PSUM") as hps,
            tc.tile_pool(name="lps", bufs=2, space="PSUM") as lps,
            tc.tile_pool(name="wps_p", bufs=1, space="PSUM") as wps_p,
        ):
            # ---- resident inputs -------------------------------------
            bf_sb = singles.tile([128, BFW], bf)
            f32_sb = singles.tile([128, FW], f32)
            fA = singles.tile([128, NR, WP], bf)
            fB = singles.tile([128, NR, WP], bf)
            f4c = singles.tile([64, NR, WP], bf)
            wsrc = singles.tile([128, 512], bf)
            # fT2h[half][p, (row,kw), (n,ci)]: transposed f4 rows 3..36 with
            # 3 horizontal shifts; a tile's 9 tap blocks are equally spaced
            # (tap stride 64) so one image's patch is a 2-free-dim AP.
            fT2h = [singles.tile([128, 34 * 3 * 64], bf, name=f"fT2h{h}")
                    for h in range(2)]
            outq = [singles.tile([128, NT * 12], f32, name=f"outq{q}")
                    for q in range(4)]
            dummy = singles.tile([1, 16], bf)
            zsrc = singles.tile([128, 128], bf)

            # weights tail first (small), then x rows in two chunks so conv
            # chunk 0 starts as soon as the first 10 rows land.
            nc.scalar.dma_start(bf_sb[:, _XW:], bfin[:, _XW:])
            nc.scalar.dma_start(bf_sb[:, 0:10 * WP], bfin[:, 0:10 * WP])
            nc.scalar.dma_start(bf_sb[:, 10 * WP:_XW], bfin[:, 10 * WP:_XW])
            nc.scalar.dma_start(f32_sb[:, :], f32in[:, :])
            nc.gpsimd.memset(wsrc[:, :], 1.0)
            nc.gpsimd.memset(zsrc[:, :], 0.0)
            nc.gpsimd.memset(fA[:, :, :], 0.0)
            nc.gpsimd.memset(fB[:, :, :], 0.0)

            # HAM warm-up: dense full-array matmuls on dummy data so the PE
            # clock gate opens (cold 1.2 GHz -> warm 2.4 GHz) before and
            # during the quadrant-packed conv (masked MMs may not register
            # as PE activity).
            wps = wps_p.tile([128, 512], f32)
            for i in range(24):
                nc.tensor.matmul(wps[:, :], wsrc[:, 0:128], wsrc[:, :],
                                 start=True, stop=True)

            # warm ACT's vector clock (1 wait per op) so conv relu-copies
            # only ever wait on PE.
            nc.scalar.copy(dummy[0:1, 0:1], bf_sb[0:1, 0:1])
            nc.scalar.copy(dummy[0:1, 1:2], f32_sb[0:1, 0:1])
            nc.scalar.copy(dummy[0:1, 2:3], fA[0:1, 0:1, 0:1])
            nc.scalar.copy(dummy[0:1, 3:4], fB[0:1, 0:1, 0:1])

            x_sb = bf_sb[:, 0:_XW].rearrange("p (r w) -> p r w", w=WP)
            cw_sb = bf_sb[:, _CWO:_CWO + 576].rearrange(
                "p (l t o) -> p l t o", t=9, o=16)
            w2p_sb = bf_sb[:, _W2O:_W2O + 864].rearrange(
                "p (j c) -> p j c", c=432)
            w1_sb = bf_sb[0:3, _W1O:_W1O + 256]
            b2p_sb = bf_sb[0:1, _B2O:_B2O + 432]
            ones_sb = bf_sb[0:1, _ONO:_ONO + 128]
            cb_sb = f32_sb[:, 0:4]
            b1_sb = f32_sb[:, 4:6]
            shift_sb = f32_sb[:, 6:6 + NT * 12]

            # ---- conv chain ------------------------------------------
            # l: 0:x->fA  1:fA->fB  2:fB->fA  3:fA->fB, then fB->f4c
            fins = [x_sb, fA, fB, fA]
            fouts = [fA, fB, fA, fB]
            for l in range(4):
                K = 3 if l == 0 else 16
                fin, fout = fins[l], fouts[l]
                for ch in range(19):
                    r0 = 1 + 2 * ch
                    ps = cps.tile([128, 2, 256], f32, tag="convps")
                    # full-array zeroing matmul opens the chunk's group: all
                    # 128 partitions initialized, and an unmasked MM per
                    # chunk keeps the PE HAM clock-gate open (quadrant-
                    # masked MMs don't register as PE activity).
                    nc.tensor.matmul(ps[:, :, :], zsrc[:, :],
                                     wsrc[:, 0:512], start=True, stop=False)
                    for tap in range(9):
                        kh, kw = tap // 3, tap % 3
                        for n in range(4):
                            nc.tensor.matmul(
                                ps[32 * n:32 * n + 16, :, :],
                                cw_sb[32 * n:32 * n + K, l, tap, :],
                                fin[32 * n:32 * n + K,
                                    r0 + kh - 1:r0 + kh + 1,
                                    kw:kw + 256],
                                start=False, stop=False,
                                tile_position=(32 * n, 32 * n),
                            )
                    # full-array +0 closes the group across all partitions
                    nc.tensor.matmul(ps[:, 0:1, 0:1], zsrc[:, :],
                                     wsrc[:, 0:1], start=False, stop=True)
                    nc.scalar.activation(
                        fout[:, r0:r0 + 2, 1:257], ps[:, :, :],
                        mybir.ActivationFunctionType.Relu,
                        bias=cb_sb[:, l:l + 1], scale=1.0)

            # compact (32n+ci) -> contiguous 64 partitions for the xbar
            for n in range(4):
                nc.scalar.dma_start(
                    out=f4c[16 * n:16 * n + 16, :, :],
                    in_=fB[32 * n:32 * n + 16, :, :])

            # warm SP's clock on the 4 compaction DMAs (1 wait each)
            for n in range(4):
                nc.sync.dma_start(out=dummy[0:1, 4 + n:5 + n],
                                  in_=f4c[16 * n:16 * n + 1, 0:1, 0:1])

            # ---- im2col: shared row-transpose cache ------------------
            for r in range(34):
                for hf in range(2):
                    for kw in range(3):
                        nc.sync.dma_start_transpose(
                            out=fT2h[hf][:, (3 * r + kw) * 64:
                                         (3 * r + kw + 1) * 64],
                            in_=f4c[:, r + 3, 128 * hf + kw:
                                    128 * hf + kw + 128])
            fT2v = [t.rearrange("p (t x) -> p t x", x=64) for t in fT2h]

            # warm DVE's clock across the HWDGE sem lanes (recent
            # transposes cover the lanes; 1 wait per touch op).
            for k in range(9):
                r, hf, kw = 33 - (k // 6), (k // 3) % 2, k % 3
                nc.vector.tensor_copy(dummy[0:1, 13 + k % 3:14 + k % 3],
                                      fT2v[hf][0:1, 3 * r + kw, 0:1])

            # ---- per-q: h MLP, local weights, einsum -----------------
            mul, add = mybir.AluOpType.mult, mybir.AluOpType.add
            for q in range(4):
                outq_v = outq[q].rearrange("p (n c t) -> p c n t", n=4, c=3)
                for pc in range(PCH):
                    pos_t = pos_p.tile([3, 1024], bf, tag="pos")
                    nc.scalar.dma_start(
                        pos_t[:, :], post[q, :, pc * 1024:(pc + 1) * 1024])
                    hT = ht_p.tile([128, 2, 1024], bf, tag="ht")
                    for jh in range(2):
                        for hf in range(2):
                            hp = hps.tile([128, 512], f32, tag="hps")
                            nc.tensor.matmul(
                                hp[:, :],
                                w1_sb[:, jh * 128:(jh + 1) * 128],
                                pos_t[:, hf * 512:(hf + 1) * 512],
                                start=True, stop=True)
                            nc.scalar.activation(
                                hT[:, jh, hf * 512:(hf + 1) * 512], hp[:, :],
                                mybir.ActivationFunctionType.Relu,
                                bias=b1_sb[:, jh:jh + 1], scale=1.0)
                    # keep-warm pulse, dependency-tied to this chunk's hT
                    nc.tensor.matmul(wps[:, 0:64], zsrc[:, :],
                                     hT[:, 0, 0:64], start=True, stop=True)
                    for tl in range(8):
                        t = pc * 8 + tl
                        r0, hf = t // 2, t % 2
                        lwp = lps.tile([128, 3, 9, 16], f32, tag="lwp")
                        for jh in range(2):
                            nc.tensor.matmul(
                                lwp[:, :, :, :],
                                hT[:, jh, tl * 128:(tl + 1) * 128],
                                w2p_sb[:, jh, :],
                                start=(jh == 0), stop=False)
                        nc.tensor.matmul(
                            lwp[:, :, :, :], ones_sb[:, :], b2p_sb[:, :],
                            start=False, stop=True)
                        lws = lws_p.tile([128, 3, 9, 16], bf, tag="lws")
                        nc.scalar.activation(
                            lws[:, :, :, :], lwp[:, :, :, :],
                            mybir.ActivationFunctionType.Copy)
                        # products then per-(c,n) segment reduce
                        in0v = fT2v[hf][:, 3 * r0:3 * r0 + 9, :].rearrange(
                            "p t (n x) -> p n t x", n=4)
                        prod = prod_p.tile([128, 3, 4, 9, 16], bf,
                                           tag="prod")
                        for c in range(3):
                            eng = nc.gpsimd if c == 2 else nc.vector
                            eng.tensor_tensor(
                                prod[:, c, :, :, :], in0v,
                                lws[:, c, :, :].unsqueeze(1).broadcast_to(
                                    (128, 4, 9, 16)),
                                mul)
                        nc.vector.tensor_reduce(
                            out=outq_v[:, :, :, t],
                            in_=prod.rearrange("p c n t x -> p (c n) (t x)"),
                            axis=mybir.AxisListType.X, op=add)
                # add_mean: +255*RGB_MEAN[c] to every output element
                nc.vector.tensor_add(outq[q][:, :], outq[q][:, :],
                                     shift_sb)

            # ---- writeback -------------------------------------------
            for q in range(4):
                nc.gpsimd.dma_start(out=outd[q], in_=outq[q][:, :])
    _legalize_waits(nc)
    return nc


def _get_nc():
    global _NC
    if _NC is None:
        _NC = _build_program()
    return _NC


def _prep_inputs(x, pos_mat, c0w, c0b, c1w, c1b, c2w, c2b, c3w, c3b,
                 w1, b1, w2, b2):
    """Host-side packing of per-core input dicts."""
    x = np.asarray(x, np.float32)
    pos = np.asarray(pos_mat, np.float32).reshape(-1, 3)

    # conv weights: cw[32n+ci, l, kh*3+kw, co]
    cwp = np.zeros((128, 4, 9, 16), np.float32)
    cbp = np.zeros((128, 4), np.float32)
    for l, (wl, bl) in enumerate(((c0w, c0b), (c1w, c1b),
                                  (c2w, c2b), (c3w, c3b))):
        wl = np.asarray(wl, np.float32)          # (co, ci, 3, 3)
        K = wl.shape[1]
        t = wl.transpose(1, 2, 3, 0).reshape(K, 9, 16)   # (ci, tap, co)
        for n in range(4):
            cwp[32 * n:32 * n + K, l] = t
            cbp[32 * n:32 * n + 16, l] = np.asarray(bl, np.float32)

    w1 = np.asarray(w1, np.float32)              # (3, 256)
    b1p = np.asarray(b1, np.float32).reshape(2, 128).T.copy()  # [j, jh]

    # w2 columns: orig (s=ci*9+tap, c) -> permuted (c, tap, ci)
    w2 = np.asarray(w2, np.float32).reshape(256, 16, 9, 3)     # j, ci, tap, c
    w2pm = w2.transpose(0, 3, 2, 1).reshape(256, 432)          # j,(c,tap,ci)
    w2pk = w2pm.reshape(2, 128, 432).astype(BF16)              # [jh, j, 432]
    w2pk = np.ascontiguousarray(w2pk.transpose(1, 0, 2))       # [j, jh, 432]
    b2 = np.asarray(b2, np.float32).reshape(16, 9, 3)
    b2pk = b2.transpose(2, 1, 0).reshape(432)                  # (c, tap, ci)

    # pos rows ordered (h, si, w, sj); per-core chunk -> (q, 3, NPIX)
    posr = pos.reshape(Himg, 2, Wimg, 2, 3)

    # bf16 tail shared by all cores: w1 | b2p | ones
    w1pad = np.zeros((128, 256), np.float32)
    w1pad[0:3] = w1
    b2pad = np.zeros((128, 432), np.float32)
    b2pad[0] = b2pk
    onespad = np.zeros((128, 128), np.float32)
    onespad[0] = 1.0

    # f32 pack: [cb | b1c | mean-shift]
    f32pk = np.zeros((128, FW), np.float32)
    f32pk[:, 0:4] = cbp
    f32pk[:, 4:6] = b1p
    shift = np.zeros(NT * 12, np.float32)
    for n in range(4):
        for c in range(3):
            shift[(n * 3 + c) * NT:(n * 3 + c + 1) * NT] = \
                RGB_RANGE * RGB_MEAN[c]
    f32pk[:, 6:] = shift

    in_maps = []
    for core in range(NCORES):
        h0 = core * ROWS
        xh = np.zeros((128, NR, WP), np.float32)
        lo, hi = h0 - HALO, h0 + ROWS + HALO
        slo, shi = max(lo, 0), min(hi, Himg)
        for n in range(4):
            xh[32 * n:32 * n + 3, slo - lo:shi - lo, 1:257] = \
                x[n, :, slo:shi, :]
        bfpk = np.concatenate(
            [xh.reshape(128, -1), cwp.reshape(128, -1),
             w2pk.reshape(128, -1).astype(np.float32),
             w1pad, b2pad, onespad], axis=1)
        pc = posr[h0:h0 + ROWS].transpose(1, 3, 4, 0, 2)  # si,sj,3,h,w
        pc = pc.reshape(2, 2, 3, NPIX).reshape(4, 3, NPIX)
        in_maps.append({
            "bfin": bfpk.astype(BF16),
            "f32in": f32pk,
            "post": np.ascontiguousarray(pc).astype(BF16),
        })
    return in_maps


def _unpack_core_output(raw):
    """[4(q), 128(p), 12*NT] f32 -> (4, 3, 2*ROWS, 2*Wimg)."""
    a = np.asarray(raw, np.float32).reshape(2, 2, 128, 4, 3, NT // 2, 2)
    # (si, sj, p, n, c, r0, hf) -> (n, c, r0, si, hf, p, sj)
    return a.transpose(3, 4, 5, 0, 6, 2, 1).reshape(4, 3, 2 * ROWS,
                                                    2 * Wimg)


LAST_RESULTS = None
TRACE = False


def kernel(**inputs):
    global LAST_RESULTS
    nc = _get_nc()
    in_maps = _prep_inputs(**inputs)
    res = run_bass_kernel_spmd(nc, in_maps, core_ids=list(range(NCORES)),
                               trace=TRACE)
    LAST_RESULTS = res
    out = np.concatenate(
        [_unpack_core_output(res.results[i]["out"]) for i in range(NCORES)],
        axis=2)
    return out.astype(np.float32)
